# revision 5
# baseline (speedup 1.0000x reference)
"""MoE (8 experts, top-2 routing) kernel for Trainium2 — hidden-dim-sharded
(tensor-parallel) across 8 NeuronCores, all matmuls in bf16.

Why hidden-shard instead of expert-parallel: with one expert per core the
slowest core pads its token group to the global max (1152 of a 1024 mean),
wasting ~12% of the PE. Sharding the H=4096 hidden dim instead gives every
core a 512-wide slice of ALL 8 experts' W1/W2, so all cores do the exact
same amount of work (the full 2T = 8192 routed (token, expert) pairs at
1/8 the hidden width each), with zero token padding: both matmuls keep
tokens on the moving dim, which can be any size.

Per core c (h-slice c*512..(c+1)*512), per expert e (cnt_e tokens, exact):
  mm1: hT[hc*128:(hc+1)*128, tok] = relu(W1_slice^T @ x^T + b1)  (h on
       partitions, 4 h-chunks, contraction D=1024 via 8 chained matmuls)
  mm2: yT[dt*128:(dt+1)*128, tok] += W2_chunk^T @ hT_chunk       (d on
       partitions, 8 d-tiles, contraction 512 via 4 chained matmuls)
The host computes the gate/top-2 (replicated small gate), groups tokens
expert-major, sums the 8 partial yT outputs, applies the combine weight and
b2, and scatter-adds back to token order. Exactness: out = w*(y_dev) +
w*b2, so folding b2 on the host is exact.

Token chunks of <=512 (PSUM bank width) are software-pipelined: mm2 of
chunk i is emitted after mm1 of chunk i+1, so the PE never waits on the
scalar-engine relu. All input DMA rides the SP (sync) HWDGE queue — the
one that comes up first after the boot preamble — with weight halves
interleaved between token chunks so the x stream never starves; y output
DMA gets a dedicated queue (gpsimd) so SBUF staging buffers recycle with
low latency. A 12-matmul PE warm-up covers the queue's ~5us cold-start +
first fills and ramps the clock out of the low p-state.
"""

import numpy as np
import ml_dtypes

P = 128
D = 1024
H = 4096
E = 8
TOPK = 2
DK = D // P        # 8 contraction chunks for mm1
HS = H // E        # 512 hidden units per core
HC = HS // P       # 4 h-chunks per core
DT = D // P        # 8 output d-tiles
CH = 512           # max token chunk (PSUM bank = 512 fp32)


def _chunk_items(cnts):
    """Split each expert's token count into chunks <=CH tokens.
    Returns [(e, global_off, tsz, first_of_e)], expert-major order.
    Expert 0 leads with a small chunk so the first matmul can start right
    after the first x block lands; the final chunk is kept small so the
    last PSUM->SBUF->DRAM drain exposes almost no tail."""
    items = []
    off = 0
    for e, cnt in enumerate(cnts):
        if cnt == 0:
            continue
        sizes = []
        rem = cnt
        if e == 0 and cnt > 360:
            sizes.append(224)
            rem -= 224
        n = -(-rem // CH)
        base, r = divmod(rem, n)
        sizes += [base + 1] * r + [base] * (n - r)
        for k, s in enumerate(sizes):
            items.append((e, off, s, k == 0))
            off += s
    e, o, s, f = items[-1]
    if s > 224 and not f:
        items[-1] = (e, o, s - 128, f)
        items.append((e, o + s - 128, 128, False))
    return items


def _build_program(cnts):
    import concourse.mybir as mybir
    import concourse.tile as tile
    from concourse import bacc

    f32 = mybir.dt.float32
    bf16 = mybir.dt.bfloat16
    Relu = mybir.ActivationFunctionType.Relu
    TOT = sum(cnts)
    items = _chunk_items(cnts)
    n_items = len(items)
    first_item = {}
    for i, (e, _, _, first) in enumerate(items):
        if first:
            first_item[e] = i
    experts_seq = sorted(first_item, key=first_item.get)

    nc = bacc.Bacc(
        "TRN2",
        target_bir_lowering=False,
        debug=False,
        enable_asserts=True,
        num_devices=E,
    )
    xg_d = nc.dram_tensor("xg", [P, DK, TOT], bf16, kind="ExternalInput").ap()
    w1_d = nc.dram_tensor("w1", [P, DK, E * HS], bf16, kind="ExternalInput").ap()
    w2_d = nc.dram_tensor("w2", [P, E * HC, D], bf16, kind="ExternalInput").ap()
    b1_d = nc.dram_tensor("b1", [P, E * HC], f32, kind="ExternalInput").ap()
    y_d = nc.dram_tensor("y", [P, DT, TOT], bf16, kind="ExternalOutput").ap()

    with tile.TileContext(nc) as tc:
        with (
            tc.tile_pool(name="const", bufs=1) as const,
            tc.tile_pool(name="w1p", bufs=3) as w1p,
            tc.tile_pool(name="w2p", bufs=3) as w2p,
            tc.tile_pool(name="xgp", bufs=4) as xgp,
            tc.tile_pool(name="htp", bufs=3) as htp,
            tc.tile_pool(name="ysp", bufs=4) as ysp,
            tc.tile_pool(name="php", bufs=2, space="PSUM") as php,
            tc.tile_pool(name="pyp", bufs=6, space="PSUM") as pyp,
        ):
            w1_tiles = {}
            w2_tiles = {}
            ht_tiles = {}

            # weight-piece emitter: halves, interleaved between xg chunks on
            # the same (sync) queue so the token stream never starves
            def w1_piece(e, half):
                if e not in w1_tiles:
                    w1_tiles[e] = w1p.tile([P, DK, HS], bf16, tag="w1", name=f"w1_{e}")
                lo = half * (HS // 2)
                nc.sync.dma_start(
                    w1_tiles[e][:, :, lo:lo + HS // 2],
                    w1_d[:, :, e * HS + lo:e * HS + lo + HS // 2],
                )

            def w2_piece(e, half):
                if e not in w2_tiles:
                    w2_tiles[e] = w2p.tile([P, HC, D], bf16, tag="w2", name=f"w2_{e}")
                lo = half * (D // 2)
                nc.sync.dma_start(
                    w2_tiles[e][:, :, lo:lo + D // 2],
                    w2_d[:, e * HC:(e + 1) * HC, lo:lo + D // 2],
                )

            # schedule: expert k's w1 halves two items before its first
            # chunk, w2 halves one item before (mm2 lags mm1 by one item)
            pieces = {i: [] for i in range(n_items)}
            e0 = experts_seq[0]
            for e in experts_seq[1:]:
                f = first_item[e]
                pieces[max(1, f - 2)] += [(w1_piece, e, 0), (w1_piece, e, 1)]
                pieces[max(2, f - 1)] += [(w2_piece, e, 0), (w2_piece, e, 1)]

            b1t = const.tile([P, E * HC], f32)

            def load_xg(i, off, tsz):
                t = xgp.tile([P, DK, CH], bf16, tag="xg", name=f"xg_{i}")
                nc.sync.dma_start(t[:, :, 0:tsz], xg_d[:, :, off:off + tsz])
                return t

            xg_tiles = {}

            def emit_dma(i):
                e, off, tsz, first = items[i]
                xg_tiles[i] = load_xg(i, off, tsz)
                if i == 0:
                    nc.sync.dma_start(b1t[:], b1_d[:])
                    for q in range(1, 4):   # quarters 1-3 (0 went first)
                        nc.sync.dma_start(
                            w1_tiles[e0][:, :, q * P:(q + 1) * P],
                            w1_d[:, :, e0 * HS + q * P:e0 * HS + (q + 1) * P],
                        )
                if i == 1:
                    w2_piece(e0, 0)
                    w2_piece(e0, 1)
                for fn, e2, half in pieces[i]:
                    fn(e2, half)

            def mm1(i):
                e, off, tsz, first = items[i]
                xgt = xg_tiles.pop(i)
                w1t = w1_tiles[e]
                htt = htp.tile([P, HC, CH], bf16, tag="ht", name=f"ht_{i}")
                ht_tiles[i] = htt
                for hc in range(HC):
                    ph = php.tile([P, CH], f32, tag="ph")
                    for dk in range(DK):
                        nc.tensor.matmul(
                            ph[:, 0:tsz],
                            w1t[:, dk, hc * P:(hc + 1) * P],
                            xgt[:, dk, 0:tsz],
                            start=(dk == 0),
                            stop=(dk == DK - 1),
                        )
                    nc.scalar.activation(
                        htt[:, hc, 0:tsz], ph[:, 0:tsz], Relu,
                        bias=b1t[:, e * HC + hc:e * HC + hc + 1],
                    )

            def mm2(i):
                e, off, tsz, first = items[i]
                htt = ht_tiles.pop(i)
                w2t = w2_tiles[e]
                yst = ysp.tile([P, DT, CH], bf16, tag="ys")
                for dt in range(DT):
                    py = pyp.tile([P, CH], f32, tag="py")
                    for hc in range(HC):
                        nc.tensor.matmul(
                            py[:, 0:tsz],
                            w2t[:, hc, dt * P:(dt + 1) * P],
                            htt[:, hc, 0:tsz],
                            start=(hc == 0),
                            stop=(hc == HC - 1),
                        )
                    if dt < 5:
                        nc.vector.tensor_copy(yst[:, dt, 0:tsz], py[:, 0:tsz])
                    else:
                        nc.scalar.copy(yst[:, dt, 0:tsz], py[:, 0:tsz])
                nc.gpsimd.dma_start(y_d[:, :, off:off + tsz], yst[:, :, 0:tsz])

            # first w1 quarter leads the queue, then the first x chunk
            w1_tiles[e0] = w1p.tile([P, DK, HS], bf16, tag="w1", name="w1_first")
            nc.sync.dma_start(
                w1_tiles[e0][:, :, 0:P], w1_d[:, :, e0 * HS:e0 * HS + P]
            )

            # PE warm-up on a zeroed tile (borrowing a php PSUM bank): keeps
            # the PE busy through the DMA queue cold-start and ramps the
            # clock out of the low p-state before the first real matmul.
            warm = const.tile([P, P], f32)
            nc.any.memset(warm[:], 0.0)
            pw = php.tile([P, CH], f32, tag="ph")
            for _ in range(12):
                nc.tensor.matmul(pw[:, 0:P], warm[:], warm[:], start=True, stop=True)

            for i in range(n_items):
                emit_dma(i)
                mm1(i)
                if i > 0:
                    mm2(i - 1)
            mm2(n_items - 1)
    nc.compile()
    return nc, items


def _route(x, Wg, bg):
    """Host gate: softmax over experts + stable top-2 (mirrors jax.lax.top_k
    tie-breaking: lowest index first)."""
    logits = x @ Wg + bg
    mx = logits.max(axis=1, keepdims=True)
    ex = np.exp(logits - mx)
    gate = ex / ex.sum(axis=1, keepdims=True)
    top2 = np.argsort(-gate, axis=1, kind="stable")[:, :TOPK]
    return gate, top2


def kernel(x, Wg, bg, W1, b1, W2, b2):
    from concourse.bass_utils import run_bass_kernel_spmd

    bf = ml_dtypes.bfloat16
    x = np.asarray(x, np.float32)
    Wg = np.asarray(Wg, np.float32)
    bg = np.asarray(bg, np.float32)
    W1 = np.asarray(W1, np.float32)
    b1 = np.asarray(b1, np.float32)
    W2 = np.asarray(W2, np.float32)
    b2 = np.asarray(b2, np.float32)
    Ttok = x.shape[0]

    gate, top2 = _route(x, Wg, bg)
    expert_idx = [np.nonzero((top2 == e).any(axis=1))[0] for e in range(E)]
    cnts = [len(s) for s in expert_idx]
    TOT = sum(cnts)
    order = np.concatenate([s for s in expert_idx if len(s)])
    offs = np.cumsum([0] + cnts)

    nc, _items = _build_program(cnts)

    # xg: x^T gathered expert-major, D-chunk tiled: xg[p, dk, j] =
    # x[order[j], dk*128 + p].  Identical for every core.
    xg = np.ascontiguousarray(
        x[order].astype(bf).T.reshape(DK, P, TOT).transpose(1, 0, 2)
    )
    W1b = W1.astype(bf)
    W2b = W2.astype(bf)
    in_maps = []
    for c in range(E):
        # w1[p, dk, e*512 + h] = W1[e, dk*128+p, c*512+h]
        w1c = np.ascontiguousarray(
            W1b[:, :, c * HS:(c + 1) * HS]
            .reshape(E, DK, P, HS).transpose(2, 1, 0, 3).reshape(P, DK, E * HS)
        )
        # w2[p, e*4+hc, d] = W2[e, c*512 + hc*128 + p, d]
        w2c = np.ascontiguousarray(
            W2b[:, c * HS:(c + 1) * HS, :]
            .reshape(E, HC, P, D).transpose(2, 0, 1, 3).reshape(P, E * HC, D)
        )
        # b1s[p, e*4+hc] = b1[e, c*512 + hc*128 + p]
        b1c = np.ascontiguousarray(
            b1[:, c * HS:(c + 1) * HS].reshape(E, HC, P).transpose(2, 0, 1)
            .reshape(P, E * HC).astype(np.float32)
        )
        in_maps.append({"xg": xg, "w1": w1c, "w2": w2c, "b1": b1c})

    results = run_bass_kernel_spmd(nc, in_maps, core_ids=list(range(E))).results

    # Sum the 8 partial yT, apply combine weights, scatter back to tokens.
    acc = np.zeros((P, DT, TOT), np.float32)
    for c in range(E):
        acc += results[c]["y"].astype(np.float32)
    yT = acc.transpose(1, 0, 2).reshape(D, TOT)   # yT[d, j]
    out = np.zeros((Ttok, D), np.float32)
    for e in range(E):
        idx = expert_idx[e]
        if len(idx) == 0:
            continue
        blk = yT[:, offs[e]:offs[e + 1]].T
        out[idx] += gate[idx, e:e + 1] * blk
    # b2 contribution, folded on the host (exact: w*y device + w*b2 here)
    mask = np.zeros((Ttok, E), np.float32)
    np.put_along_axis(mask, top2, 1.0, axis=1)
    out += (gate * mask) @ b2
    return out


# revision 9
# speedup vs baseline: 1.1792x; 1.1792x over previous
"""MoE (8 experts, top-2 routing) kernel for Trainium2 — hidden-dim-sharded
(tensor-parallel) across 8 NeuronCores, all matmuls in bf16.

Why hidden-shard instead of expert-parallel: with one expert per core the
slowest core pads its token group to the global max, wasting ~12% of the
PE. Sharding the H=4096 hidden dim instead gives every core a 512-wide
slice of ALL 8 experts' W1/W2, so all cores do the exact same amount of
work (the full 2T = 8192 routed (token, expert) pairs at 1/8 the hidden
width each), with zero token padding: both matmuls keep tokens on the
moving dim, which can be any size.

Per core c (h-slice c*512..(c+1)*512), per expert e (cnt_e tokens, exact):
  mm1: hT[hc*128:(hc+1)*128, tok] = relu(W1_slice^T @ x^T + b1)  (h on
       partitions, 4 h-chunks, contraction D=1024 via 8 chained matmuls)
  mm2: yT[dt*128:(dt+1)*128, tok] += W2_chunk^T @ hT_chunk       (d on
       partitions, 8 d-tiles, contraction 512 via 4 chained matmuls)
The host computes the gate/top-2 (replicated small gate), groups tokens
expert-major, sums the 8 partial yT outputs, applies the combine weight and
b2, and scatter-adds back to token order. Exactness: out = w*(y_dev) +
w*b2, so folding b2 on the host is exact.

Token chunks of <=512 (PSUM bank width) are software-pipelined: mm2 of
chunk i is emitted after mm1 of chunk i+1, so the PE never waits on the
scalar-engine relu. All input DMA rides the SP (sync) HWDGE queue — the
one that comes up first after the boot preamble — with weight HALVES
interleaved between token chunks so the x stream never starves during the
pipeline fill; y output DMA gets a dedicated queue (gpsimd) so SBUF
staging buffers recycle with low latency. A 12-matmul PE warm-up covers
the queue's ~5us cold-start + first fills and ramps the clock out of the
low p-state.
"""

import numpy as np
import ml_dtypes

P = 128
D = 1024
H = 4096
E = 8
TOPK = 2
DK = D // P        # 8 contraction chunks for mm1
HS = H // E        # 512 hidden units per core
HC = HS // P       # 4 h-chunks per core
DT = D // P        # 8 output d-tiles
CH = 512           # max token chunk (PSUM bank = 512 fp32)


def _chunk_items(cnts):
    """Split each expert's token count into chunks <=CH tokens.
    Returns [(e, global_off, tsz, first_of_e)], expert-major order.
    Expert 0 leads with a small chunk so the first matmul can start right
    after the first x block lands; the final chunk is kept small so the
    last PSUM->SBUF->DRAM drain exposes almost no tail."""
    items = []
    off = 0
    for e, cnt in enumerate(cnts):
        if cnt == 0:
            continue
        sizes = []
        rem = cnt
        if e == 0 and cnt > 360:
            sizes.append(224)
            rem -= 224
        n = -(-rem // CH)
        base, r = divmod(rem, n)
        sizes += [base + 1] * r + [base] * (n - r)
        for k, s in enumerate(sizes):
            items.append((e, off, s, k == 0))
            off += s
    e, o, s, f = items[-1]
    if s > 224 and not f:
        items[-1] = (e, o, s - 128, f)
        items.append((e, o + s - 128, 128, False))
    return items


def _build_program(cnts):
    import concourse.mybir as mybir
    import concourse.tile as tile
    from concourse import bacc

    f32 = mybir.dt.float32
    bf16 = mybir.dt.bfloat16
    Relu = mybir.ActivationFunctionType.Relu
    TOT = sum(cnts)
    items = _chunk_items(cnts)
    n_items = len(items)
    first_item = {}
    for i, (e, _, _, first) in enumerate(items):
        if first:
            first_item[e] = i
    experts_seq = sorted(first_item, key=first_item.get)

    nc = bacc.Bacc(
        "TRN2",
        target_bir_lowering=False,
        debug=False,
        enable_asserts=True,
        num_devices=E,
    )
    xg_d = nc.dram_tensor("xg", [P, DK, TOT], bf16, kind="ExternalInput").ap()
    w1_d = nc.dram_tensor("w1", [P, DK, E * HS], bf16, kind="ExternalInput").ap()
    w2_d = nc.dram_tensor("w2", [P, E * HC, D], bf16, kind="ExternalInput").ap()
    b1_d = nc.dram_tensor("b1", [P, E * HC], f32, kind="ExternalInput").ap()
    y_d = nc.dram_tensor("y", [P, DT, TOT], bf16, kind="ExternalOutput").ap()

    with tile.TileContext(nc) as tc:
        with (
            tc.tile_pool(name="const", bufs=1) as const,
            tc.tile_pool(name="w1p", bufs=3) as w1p,
            tc.tile_pool(name="w2p", bufs=2) as w2p,
            tc.tile_pool(name="xgp", bufs=4) as xgp,
            tc.tile_pool(name="htp", bufs=3) as htp,
            tc.tile_pool(name="ysp", bufs=3) as ysp,
            tc.tile_pool(name="php", bufs=3, space="PSUM") as php,
            tc.tile_pool(name="pyp", bufs=4, space="PSUM") as pyp,
            tc.tile_pool(name="pwp", bufs=1, space="PSUM") as pwp,
        ):
            w1_tiles = {}
            w2_tiles = {}
            ht_tiles = {}
            xg_tiles = {}

            def w1_piece(e, half):
                if e not in w1_tiles:
                    w1_tiles[e] = w1p.tile([P, DK, HS], bf16, tag="w1",
                                           name=f"w1_{e}")
                lo = half * (HS // 2)
                nc.sync.dma_start(
                    w1_tiles[e][:, :, lo:lo + HS // 2],
                    w1_d[:, :, e * HS + lo:e * HS + lo + HS // 2],
                )

            def w2_piece(e, half):
                if e not in w2_tiles:
                    w2_tiles[e] = w2p.tile([P, HC, D], bf16, tag="w2",
                                           name=f"w2_{e}")
                lo = half * (D // 2)
                nc.sync.dma_start(
                    w2_tiles[e][:, :, lo:lo + D // 2],
                    w2_d[:, e * HC:(e + 1) * HC, lo:lo + D // 2],
                )

            # weight-piece schedule: expert k's w1 halves land two/one items
            # before its first chunk; its w2 halves at first/first+1 (mm2
            # lags mm1 by one item, so both stay ahead of first use while
            # never letting >1 weight piece delay the next x chunk).
            e0 = experts_seq[0]
            pieces = {i: [] for i in range(n_items)}
            # both e0 w2 halves must precede mm2(0), which is emitted right
            # after mm1(1) — so they go with item 1's DMA batch
            pieces[min(1, n_items - 1)].append((w2_piece, e0, 0))
            pieces[min(1, n_items - 1)].append((w2_piece, e0, 1))
            for e in experts_seq[1:]:
                f = first_item[e]
                pieces[max(1, f - 2)].append((w1_piece, e, 0))
                pieces[max(1, f - 1)].append((w1_piece, e, 1))
                pieces[f].append((w2_piece, e, 0))
                pieces[min(f + 1, n_items - 1)].append((w2_piece, e, 1))

            b1t = const.tile([P, E * HC], f32)

            def emit_dma(i):
                e, off, tsz, first = items[i]
                t = xgp.tile([P, DK, CH], bf16, tag="xg", name=f"xg_{i}")
                nc.sync.dma_start(t[:, :, 0:tsz], xg_d[:, :, off:off + tsz])
                xg_tiles[i] = t
                if i == 0:
                    nc.sync.dma_start(b1t[:], b1_d[:])
                    for q in range(1, 4):   # e0 w1 quarters 1-3 (0 led off)
                        nc.sync.dma_start(
                            w1_tiles[e0][:, :, q * P:(q + 1) * P],
                            w1_d[:, :, e0 * HS + q * P:e0 * HS + (q + 1) * P],
                        )
                for fn, e2, half in pieces[i]:
                    fn(e2, half)

            def mm1(i):
                e, off, tsz, first = items[i]
                xgt = xg_tiles.pop(i)
                w1t = w1_tiles[e]
                htt = htp.tile([P, HC, CH], bf16, tag="ht", name=f"ht_{i}")
                ht_tiles[i] = htt
                for hc in range(HC):
                    ph = php.tile([P, CH], f32, tag="ph")
                    for dk in range(DK):
                        nc.tensor.matmul(
                            ph[:, 0:tsz],
                            w1t[:, dk, hc * P:(hc + 1) * P],
                            xgt[:, dk, 0:tsz],
                            start=(dk == 0),
                            stop=(dk == DK - 1),
                        )
                    nc.scalar.activation(
                        htt[:, hc, 0:tsz], ph[:, 0:tsz], Relu,
                        bias=b1t[:, e * HC + hc:e * HC + hc + 1],
                    )

            def mm2(i):
                e, off, tsz, first = items[i]
                htt = ht_tiles.pop(i)
                w2t = w2_tiles[e]
                yst = ysp.tile([P, DT, CH], bf16, tag="ys", name=f"ys_{i}")
                for dt in range(DT):
                    py = pyp.tile([P, CH], f32, tag="py")
                    for hc in range(HC):
                        nc.tensor.matmul(
                            py[:, 0:tsz],
                            w2t[:, hc, dt * P:(dt + 1) * P],
                            htt[:, hc, 0:tsz],
                            start=(hc == 0),
                            stop=(hc == HC - 1),
                        )
                    if dt % 2 == 0:
                        nc.vector.tensor_copy(yst[:, dt, 0:tsz], py[:, 0:tsz])
                    else:
                        nc.scalar.copy(yst[:, dt, 0:tsz], py[:, 0:tsz])
                nc.gpsimd.dma_start(y_d[:, :, off:off + tsz], yst[:, :, 0:tsz])

            # first w1 quarter leads the input queue, then the first x chunk
            w1_tiles[e0] = w1p.tile([P, DK, HS], bf16, tag="w1",
                                    name="w1_first")
            nc.sync.dma_start(
                w1_tiles[e0][:, :, 0:P], w1_d[:, :, e0 * HS:e0 * HS + P]
            )

            # PE warm-up on a zeroed tile: keeps the PE busy through the DMA
            # queue cold-start and ramps the clock out of the low p-state.
            warm = const.tile([P, P], f32)
            nc.any.memset(warm[:], 0.0)
            pw = pwp.tile([P, P], f32, tag="pw")
            for _ in range(12):
                nc.tensor.matmul(pw[:], warm[:], warm[:], start=True, stop=True)

            for i in range(n_items):
                emit_dma(i)
                mm1(i)
                if i > 0:
                    mm2(i - 1)
            mm2(n_items - 1)
    nc.compile()
    return nc, items


def _route(x, Wg, bg):
    """Host gate: softmax over experts + stable top-2 (mirrors jax.lax.top_k
    tie-breaking: lowest index first)."""
    logits = x @ Wg + bg
    mx = logits.max(axis=1, keepdims=True)
    ex = np.exp(logits - mx)
    gate = ex / ex.sum(axis=1, keepdims=True)
    top2 = np.argsort(-gate, axis=1, kind="stable")[:, :TOPK]
    return gate, top2


def kernel(x, Wg, bg, W1, b1, W2, b2):
    from concourse.bass_utils import run_bass_kernel_spmd

    bf = ml_dtypes.bfloat16
    x = np.asarray(x, np.float32)
    Wg = np.asarray(Wg, np.float32)
    bg = np.asarray(bg, np.float32)
    W1 = np.asarray(W1, np.float32)
    b1 = np.asarray(b1, np.float32)
    W2 = np.asarray(W2, np.float32)
    b2 = np.asarray(b2, np.float32)
    Ttok = x.shape[0]

    gate, top2 = _route(x, Wg, bg)
    expert_idx = [np.nonzero((top2 == e).any(axis=1))[0] for e in range(E)]
    cnts = [len(s) for s in expert_idx]
    TOT = sum(cnts)
    order = np.concatenate([s for s in expert_idx if len(s)])
    offs = np.cumsum([0] + cnts)

    nc, _items = _build_program(cnts)

    # xg: x^T gathered expert-major, D-chunk tiled: xg[p, dk, j] =
    # x[order[j], dk*128 + p].  Identical for every core.
    xg = np.ascontiguousarray(
        x[order].astype(bf).T.reshape(DK, P, TOT).transpose(1, 0, 2)
    )
    W1b = W1.astype(bf)
    W2b = W2.astype(bf)
    in_maps = []
    for c in range(E):
        # w1[p, dk, e*512 + h] = W1[e, dk*128+p, c*512+h]
        w1c = np.ascontiguousarray(
            W1b[:, :, c * HS:(c + 1) * HS]
            .reshape(E, DK, P, HS).transpose(2, 1, 0, 3).reshape(P, DK, E * HS)
        )
        # w2[p, e*4+hc, d] = W2[e, c*512 + hc*128 + p, d]
        w2c = np.ascontiguousarray(
            W2b[:, c * HS:(c + 1) * HS, :]
            .reshape(E, HC, P, D).transpose(2, 0, 1, 3).reshape(P, E * HC, D)
        )
        # b1s[p, e*4+hc] = b1[e, c*512 + hc*128 + p]
        b1c = np.ascontiguousarray(
            b1[:, c * HS:(c + 1) * HS].reshape(E, HC, P).transpose(2, 0, 1)
            .reshape(P, E * HC).astype(np.float32)
        )
        in_maps.append({"xg": xg, "w1": w1c, "w2": w2c, "b1": b1c})

    results = run_bass_kernel_spmd(nc, in_maps, core_ids=list(range(E))).results

    # Sum the 8 partial yT, apply combine weights, scatter back to tokens.
    acc = np.zeros((P, DT, TOT), np.float32)
    for c in range(E):
        acc += results[c]["y"].astype(np.float32)
    yT = acc.transpose(1, 0, 2).reshape(D, TOT)   # yT[d, j]
    out = np.zeros((Ttok, D), np.float32)
    for e in range(E):
        idx = expert_idx[e]
        if len(idx) == 0:
            continue
        blk = yT[:, offs[e]:offs[e + 1]].T
        out[idx] += gate[idx, e:e + 1] * blk
    # b2 contribution, folded on the host (exact: w*y device + w*b2 here)
    mask = np.zeros((Ttok, E), np.float32)
    np.put_along_axis(mask, top2, 1.0, axis=1)
    out += (gate * mask) @ b2
    return out
